# revision 34
# baseline (speedup 1.0000x reference)
"""Trainium2 Bass kernel for nn_Dataset1V7Table5Redo_69741678952822 (topk_masking).

Math: the reference's set-valued +/- path expansion collapses algebraically.
Per row (N = batch*choices = 65536, D = 256):
    t1 = tanh(W1 @ x)            (128)
    t2 = tanh(W2 @ t1)           (128)
    y  = Wout @ t2               (scalar)
    a  = sum_h sob[h] * sin(2*pi*soa[h]*y/7)
    out = sign(a) * y * sigmoid(|a| - ln(5/4))

Sharding: pure data parallel over rows, 8192 rows/core on 8 cores.
Host pre-transposes x so the contraction dim lands on SBUF partitions.

Precision: single-fp16 everywhere (x, weights, activations). Host-side
float64 simulation of this exact scheme gives rel err 5.5e-4 vs the fp32
reference (gate is 2e-2): the handful of sign(a) flips land near y=0 where
the output is tiny. Optional fp16 hi/lo planes per weight matrix can be
re-enabled via the LO_* flags (each adds one PE pass per matmul).

Structure per 512-row chunk: PE does 2 L1 k-half passes + 1 L2 + 1 u pass
(+ a/4); ACT does tanh1 (paired across 2 chunks to amortize access bubbles),
tanh2 and sin/4, each writing fp16 SBUF directly. Group tails (2048 rows):
rint range reduction (MAGIC trick) on DVE, sin on ACT, then the per-row 'a'
reduction on PE. The (4,512) a / y rows are reshaped into a c-major (64,128)
finals layout with 8 tiny PE transposes per group (a-side, no DMA) and one
strided SBUF gather DMA (y-side, off the critical chain), so the drain tail
only carries PE/DVE/ACT ops plus the final output DMA.

Activation tables: Tanh and Sin co-reside only in silu_and_others; a Bacc
subclass pins table selection there so there is exactly one table load.
"""

import math
from contextlib import ExitStack

import numpy as np

import concourse.bass as bass
import concourse.tile as tile
from concourse import bacc, mybir
from concourse.hw_specs import get_activation_tables
import bass_rust as _bass_rust

F32 = mybir.dt.float32
F16 = mybir.dt.float16
I32 = mybir.dt.int32
AF = mybir.ActivationFunctionType
OP = mybir.AluOpType

N_CORES = 8
NROWS = 65536          # total rows
R = NROWS // N_CORES   # rows per core = 8192
CH = 512               # rows per chunk (one psum bank)
NCH = R // CH          # 16 chunks
NGRP = NCH // 4        # 4 groups
BLK = 2048             # xt dma block columns
NBLK = R // BLK        # 4 blocks

# optional fp16 lo-planes (one extra PE pass each where enabled)
LO_W1 = False
LO_W2 = False
LO_TW = False
LO_BS = False

MAGIC = float(np.float32(1.5 * 2 ** 23))   # fp32 round-to-nearest-int trick
TWO_PI = float(2.0 * math.pi)
LN54 = float(math.log(1.25))


def _wlayout():
    """Column layout of the packed fp16 weight tile."""
    off, lay = 0, {}
    for nm, w in (("w1h0", 128), ("w1h1", 128), ("w2h", 128), ("twh", 32),
                  ("bsh", 4)):
        lay[nm] = (off, off + w); off += w
    for flag, nm, w in ((LO_W1, "w1l0", 128), (LO_W1, "w1l1", 128),
                        (LO_W2, "w2l", 128), (LO_TW, "twl", 32),
                        (LO_BS, "bsl", 4)):
        if flag:
            lay[nm] = (off, off + w); off += w
    return lay, off


class _Bacc(bacc.Bacc):
    """Bacc whose activation-table pass may only pick silu_and_others
    (contains both Tanh and Sin) -> exactly one ACT_TABLE_LOAD."""

    def insert_act_table_loads(self):
        has_act = any(
            isinstance(i, mybir.InstActivation)
            for b in self.main_func.blocks
            for i in b.instructions
        )
        if not has_act:
            return
        tables = list(get_activation_tables(self.m.arch).items())
        masked = [
            (nm, fns if nm == "silu_and_others" else set()) for nm, fns in tables
        ]
        _bass_rust.insert_act_table_loads(self, masked)


def build_module():
    """Build + bacc-compile the (input-independent) Bass module."""
    lay, wcols = _wlayout()
    nc = _Bacc(
        "TRN2",
        target_bir_lowering=False,
        debug=False,
        enable_asserts=False,
        num_devices=N_CORES,
    )
    xh = nc.dram_tensor("xh", (2, 128, R), F16, kind="ExternalInput").ap()
    wpk = nc.dram_tensor("wpk", (128, wcols), F16, kind="ExternalInput").ap()
    # fincons cols: 0 = y scale (0.5*7/soa0), 1 = tanh bias, 2:6 = I4
    fincons = nc.dram_tensor("fincons", (128, 6), F32, kind="ExternalInput").ap()
    out = nc.dram_tensor("out", (R,), F32, kind="ExternalOutput").ap()

    with tile.TileContext(nc) as tc, ExitStack() as ctx:
        consts = ctx.enter_context(tc.tile_pool(name="consts", bufs=1))
        xpool = ctx.enter_context(tc.tile_pool(name="x", bufs=1))
        # PSUM budget (8 banks): z1 pairs 2x2 + z2 2 + u 2; the a-matmul and
        # its transposes reuse the drained u tile's partitions.
        z1ps = ctx.enter_context(tc.tile_pool(name="z1ps", bufs=2, space="PSUM"))
        z2ps = ctx.enter_context(tc.tile_pool(name="z2ps", bufs=2, space="PSUM"))
        ups = ctx.enter_context(tc.tile_pool(name="ups", bufs=2, space="PSUM"))
        t1p = ctx.enter_context(tc.tile_pool(name="t1p", bufs=2))
        t2p = ctx.enter_context(tc.tile_pool(name="t2p", bufs=3))
        kp = ctx.enter_context(tc.tile_pool(name="kp", bufs=2))
        vp = ctx.enter_context(tc.tile_pool(name="vp", bufs=2))
        shp = ctx.enter_context(tc.tile_pool(name="shp", bufs=2))
        rp = ctx.enter_context(tc.tile_pool(name="rp", bufs=2))
        finp = ctx.enter_context(tc.tile_pool(name="finp", bufs=1))

        wc = consts.tile([128, wcols], F16, tag="wc")
        W = {nm: wc[:, a:b] for nm, (a, b) in lay.items()}
        fcs = consts.tile([128, 6], F32, tag="fincons")
        ident4 = fcs[0:4, 2:6]
        # one x tile per block: plane k at cols [k*BLK, (k+1)*BLK)
        xt = [xpool.tile([128, 2 * BLK], F16, tag=f"xt{b}", name=f"xt{b}")
              for b in range(NBLK)]

        def xsl(c, k):
            return xt[c // 4][:, k * BLK + (c % 4) * CH:
                              k * BLK + (c % 4) * CH + CH]

        xv = xh.rearrange("k f c -> f k c")

        def xdst(b, c0, c1):
            return (xt[b][:].rearrange("f (k c) -> f k c", k=2)[:, :, c0:c1])

        # DMA transfers serialize on the shared DMA-engine pool, so issue in
        # need-order: weights first (gpsimd SWDGE), per-chunk pieces for the
        # first two blocks on SP, the last two blocks on the ACT queue (only
        # 2 configs there so the ACT sequencer stays clear for activations).
        nc.gpsimd.dma_start(wc[:], wpk)
        for c in range(8):
            nc.sync.dma_start(xdst(c // 4, (c % 4) * CH, (c % 4 + 1) * CH),
                              xv[:, :, c * CH:(c + 1) * CH])
        nc.gpsimd.dma_start(fcs[:], fincons)
        nc.scalar.dma_start(xdst(2, 0, BLK), xv[:, :, 2 * BLK:3 * BLK])
        nc.scalar.dma_start(xdst(3, 0, BLK), xv[:, :, 3 * BLK:4 * BLK])

        # finals tiles (128, 64): partition 32g+8j+(r//64), col r%64 holds
        # group-g row 512j + r
        yfin = finp.tile([128, 64], F32, tag="yfin")
        afin = finp.tile([128, 64], F32, tag="afin")

        st = {}   # per-chunk state
        pr = {}   # per-pair state (z1/t1 are batched 2 chunks per tile)
        grp = {}  # per-group state

        def s_l1(c):
            p = c // 2
            st[c] = {}
            if p == 0:
                # first pair decoupled: separate psum tiles (borrowed from
                # the z2 pool rotation) so tanh1(c0) doesn't wait on chunk 1
                pr.setdefault(0, {})[f"z1s{c}"] = z2ps.tile(
                    [128, CH], F32, tag="z2", name=f"z1s_{c}")
                z1h = pr[0][f"z1s{c}"][:]
            else:
                if c % 2 == 0:
                    pr[p] = {"z1": z1ps.tile([128, 2 * CH], F32, tag="z1",
                                             name=f"z1_{p}")}
                z1h = pr[p]["z1"][:, (c % 2) * CH:(c % 2 + 1) * CH]
            passes = [W["w1h0"], W["w1h1"]]
            if LO_W1:
                passes += [W["w1l0"], W["w1l1"]]
            for i, w in enumerate(passes):
                nc.tensor.matmul(z1h, w, xsl(c, i % 2), start=(i == 0),
                                 stop=(i == len(passes) - 1))

        def s_tanh1(p, half=None):
            d = pr[p]
            if half is None:
                d["t1"] = t1p.tile([128, 2 * CH], F16, tag="t1", name=f"t1_{p}")
                nc.scalar.activation(d["t1"][:], d["z1"][:], AF.Tanh)
            else:
                if half == 0:
                    d["t1"] = t1p.tile([128, 2 * CH], F16, tag="t1",
                                       name=f"t1_{p}")
                sl = slice(half * CH, (half + 1) * CH)
                nc.scalar.activation(d["t1"][:, sl], d[f"z1s{half}"][:],
                                     AF.Tanh)

        def s_l2(c):
            d = st[c]
            t1h = pr[c // 2]["t1"][:, (c % 2) * CH:(c % 2 + 1) * CH]
            z2 = z2ps.tile([128, CH], F32, tag="z2", name=f"z2_{c}")
            d["z2"] = z2
            nc.tensor.matmul(z2[:], W["w2h"], t1h, start=True,
                             stop=not LO_W2)
            if LO_W2:
                nc.tensor.matmul(z2[:], W["w2l"], t1h, start=False,
                                 stop=True)

        def s_tanh2(c):
            d = st[c]
            d["t2"] = t2p.tile([128, CH], F16, tag="t2", name=f"t2_{c}")
            nc.scalar.activation(d["t2"][:], d["z2"][:], AF.Tanh)

        def s_umm(c):
            d = st[c]
            g, j = c // 4, c % 4
            if j == 0:
                grp[g] = {"u": ups.tile([128, CH], F32, tag="u",
                                        name=f"u_{g}")}
            od = grp[g]["u"][32 * j:32 * (j + 1), :]
            tp = (0, 32 * j)
            nc.tensor.matmul(od, W["twh"], d["t2"][:], start=True,
                             stop=not LO_TW, tile_position=tp)
            if LO_TW:
                nc.tensor.matmul(od, W["twl"], d["t2"][:], start=False,
                                 stop=True, tile_position=tp)
            del st[c]

        def s_ga(g):
            d = grp[g]
            # u drain for the y rows {0,32,64,96}; the last group uses the
            # (then idle) ACT engine so DVE can start k/v immediately
            d["uc"] = rp.tile([128, CH], F32, tag="uc", name=f"uc_{g}")
            if g == NGRP - 1:
                nc.scalar.copy(d["uc"][:], d["u"][:])
            else:
                nc.vector.tensor_copy(d["uc"][:], d["u"][:])
            # j-major strip of the 4 y rows straight into yfin's group block
            raw4 = d["uc"][:].rearrange("(jj h) r -> h jj r", h=32)[0]
            nc.sync.dma_start(yfin[32 * g:32 * (g + 1), :], raw4)
            # range reduction: k = rint(u) (MAGIC trick), v = u - k
            d["k"] = kp.tile([128, CH], F32, tag="k", name=f"k_{g}")
            nc.vector.tensor_scalar(d["k"][:], d["u"][:], MAGIC, -MAGIC,
                                    OP.add, OP.add)
            d["v"] = vp.tile([128, CH], F32, tag="v", name=f"v_{g}")
            nc.vector.scalar_tensor_tensor(d["v"][:], d["u"][:], 0.0,
                                           d["k"][:], OP.add, OP.subtract)

        def s_gb(g):
            d = grp[g]
            d["sh"] = shp.tile([128, CH], F16, tag="sh", name=f"sh_{g}")
            nc.scalar.activation(d["sh"][:], d["v"][:], AF.Sin, scale=TWO_PI)

        def s_gc(g):
            # u is fully drained: reuse it for the a matmul (partitions 0:4)
            d = grp[g]
            a4 = d["u"][0:4, :]
            d["a4"] = a4
            nc.tensor.matmul(a4, W["bsh"], d["sh"][:], start=True,
                             stop=not LO_BS)
            if LO_BS:
                nc.tensor.matmul(a4, W["bsl"], d["sh"][:], start=False,
                                 stop=True)

        def s_gd(g):
            d = grp[g]
            ar4 = rp.tile([4, CH], F32, tag="ar4", name=f"ar4_{g}")
            nc.vector.tensor_copy(ar4[:], d["a4"])
            nc.sync.dma_start(afin[32 * g:32 * (g + 1), :], ar4[:])
            del grp[g]

        def s_fin():
            """Batched finals on the gathered (128, 64) tiles + output DMA.
            Critical chain after the last afin gather: aab -> tnh -> ot."""
            t = {}
            for nm, dt_ in (("aab", I32), ("gsn", I32), ("tnh", F32),
                            ("yv", F32), ("ysg", I32), ("ot", F32)):
                t[nm] = finp.tile([128, 64], dt_, tag=nm, name=nm)
            # off-chain ops first (only depend on yfin / afin sign bits)
            nc.vector.tensor_scalar(t["yv"][:], yfin[:], fcs[:, 0:1],
                                    None, OP.mult)
            nc.vector.tensor_scalar(t["gsn"][:], afin[:].bitcast(I32),
                                    -2 ** 31, None, OP.bitwise_and)
            nc.vector.tensor_tensor(t["ysg"][:], t["yv"][:].bitcast(I32),
                                    t["gsn"][:], OP.bitwise_xor)
            nc.vector.tensor_scalar(t["aab"][:], afin[:].bitcast(I32),
                                    0x7FFFFFFF, None, OP.bitwise_and)
            nc.scalar.activation(t["tnh"][:], t["aab"][:].bitcast(F32),
                                 AF.Tanh, scale=0.5, bias=fcs[:, 1:2])
            # ot = (tnh + 1) * ysg  == sign(a) * y * sigmoid(|a|-ln(5/4))
            nc.vector.scalar_tensor_tensor(t["ot"][:], t["tnh"][:], 1.0,
                                           t["ysg"][:].bitcast(F32),
                                           OP.add, OP.mult)
            nc.sync.dma_start(out.rearrange("(a b) -> a b", b=64),
                              t["ot"][:])

        # modulo schedule: emission order fixes per-engine FIFO order
        for t in range(NCH + 10):
            if t < NCH:
                s_l1(t)
            if t in (1, 2):      # pair 0 unpaired for fast start
                s_tanh1(0, half=t - 1)
            if t % 2 == 0 and 4 <= t <= NCH:
                s_tanh1(t // 2 - 1)
            c = t - 3
            if 0 <= c < NCH:
                s_l2(c)
            c = t - 4
            if 0 <= c < NCH:
                s_tanh2(c)
            c = t - 5
            if 0 <= c < NCH:
                s_umm(c)
            c = t - 6   # c%4==3 completes group g=c//4
            if 0 <= c < NCH and c % 4 == 3:
                s_ga(c // 4)
            c = t - 7
            if 0 <= c < NCH and c % 4 == 3:
                s_gb(c // 4)
            c = t - 8
            if 0 <= c < NCH and c % 4 == 3:
                s_gc(c // 4)
            c = t - 9
            if 0 <= c < NCH and c % 4 == 3:
                s_gd(c // 4)
        s_fin()

    nc.compile()
    return nc


_NC_CACHE = None


def _get_module():
    global _NC_CACHE
    if _NC_CACHE is None:
        _NC_CACHE = build_module()
    return _NC_CACHE


def _f16(v):
    return np.asarray(v, np.float32).astype(np.float16)


def _f16lo(v):
    v = np.asarray(v, np.float32)
    h = v.astype(np.float16)
    return (v - h.astype(np.float32)).astype(np.float16)


def prep_inputs(x, W1, W2, Wout, s1a, s1b, s2a, s2b, soa, sob):
    """Host-side prep: shard x^T per core (fp16), prepack weights."""
    lay, wcols = _wlayout()
    x = np.asarray(x, np.float32).reshape(NROWS, 256)
    W1 = np.asarray(W1, np.float64)
    W2 = np.asarray(W2, np.float64)
    wout = np.asarray(Wout, np.float64)[0]          # (128,)
    soa_v = np.asarray(soa, np.float64)[:, 0]       # (32,)
    sob_v = np.asarray(sob, np.float64)[0]          # (32,)

    # component order: y-recovery component first (a is order-invariant)
    hstar = int(np.argmax(np.abs(soa_v)))
    perm = [hstar] + [h for h in range(32) if h != hstar]
    soa_p = soa_v[perm]
    sob_p = sob_v[perm]

    w1t = np.ascontiguousarray(
        W1.reshape(128, 2, 128).transpose(1, 2, 0))     # (2,128,128) [k,f,m]
    w2t = np.ascontiguousarray(W2.T)
    tailw = np.ascontiguousarray(
        wout[:, None] * soa_p[None, :] / 7.0)           # (128,32)
    bsob = np.zeros((128, 4), np.float64)
    for j in range(4):
        bsob[32 * j:32 * (j + 1), j] = sob_p

    wpk = np.zeros((128, wcols), np.float16)
    parts = {"w1h0": _f16(w1t[0]), "w1h1": _f16(w1t[1]),
             "w2h": _f16(w2t), "twh": _f16(tailw), "bsh": _f16(bsob)}
    if LO_W1:
        parts["w1l0"] = _f16lo(w1t[0]); parts["w1l1"] = _f16lo(w1t[1])
    if LO_W2:
        parts["w2l"] = _f16lo(w2t)
    if LO_TW:
        parts["twl"] = _f16lo(tailw)
    if LO_BS:
        parts["bsl"] = _f16lo(bsob)
    for nm, (a, b) in lay.items():
        wpk[:, a:b] = parts[nm]

    # col0: y-recovery scale (0.5 from sigmoid=0.5*(1+tanh) folded in)
    # col1: tanh bias -ln(5/4)/2; cols 2:6 identity for PE transposes
    fincons = np.zeros((128, 6), np.float32)
    fincons[:, 0] = np.float32(0.5 * 7.0 / soa_p[0])
    fincons[:, 1] = np.float32(-LN54 / 2.0)
    fincons[0:4, 2:6] = np.eye(4, dtype=np.float32)

    xT = x.T.astype(np.float16)                       # (256, 65536)
    in_maps = []
    for c in range(N_CORES):
        xc = np.ascontiguousarray(xT[:, c * R:(c + 1) * R]).reshape(2, 128, R)
        in_maps.append({
            "xh": xc, "wpk": wpk, "fincons": fincons,
        })
    return in_maps


def kernel(x, W1, W2, Wout, s1a, s1b, s2a, s2b, soa, sob):
    from concourse.bass_utils import run_bass_kernel_spmd

    nc = _get_module()
    in_maps = prep_inputs(x, W1, W2, Wout, s1a, s1b, s2a, s2b, soa, sob)
    res = run_bass_kernel_spmd(nc, in_maps, core_ids=list(range(N_CORES)))
    full = np.concatenate([res.results[c]["out"] for c in range(N_CORES)])
    return full.reshape(1024, 64).astype(np.float32)


# revision 35
# speedup vs baseline: 1.1092x; 1.1092x over previous
"""Trainium2 Bass kernel for nn_Dataset1V7Table5Redo_69741678952822 (topk_masking).

Math: the reference's set-valued +/- path expansion collapses algebraically.
Per row (N = batch*choices = 65536, D = 256):
    t1 = tanh(W1 @ x)            (128)
    t2 = tanh(W2 @ t1)           (128)
    y  = Wout @ t2               (scalar)
    a  = sum_h sob[h] * sin(2*pi*soa[h]*y/7)
    out = sign(a) * y * sigmoid(|a| - ln(5/4))

Sharding: pure data parallel over rows, 8192 rows/core on 8 cores.
Host pre-transposes x so the contraction dim lands on SBUF partitions.

Precision: single-fp16 everywhere (x, weights, activations). Host-side
float64 simulation of this exact scheme gives rel err 5.5e-4 vs the fp32
reference (gate is 2e-2): the handful of sign(a) flips land near y=0 where
the output is tiny. Optional fp16 hi/lo planes per weight matrix can be
re-enabled via the LO_* flags (each adds one PE pass per matmul).

Structure per 512-row chunk: PE does 2 L1 k-half passes + 1 L2 + 1 u pass
(+ a/4); ACT does tanh1 (paired across 2 chunks to amortize access bubbles),
tanh2 and sin/4, each writing fp16 SBUF directly. Group tails (2048 rows):
rint range reduction (MAGIC trick) on DVE, sin on ACT, then the per-row 'a'
reduction on PE. The (4,512) a / y rows are reshaped into a c-major (64,128)
finals layout with 8 tiny PE transposes per group (a-side, no DMA) and one
strided SBUF gather DMA (y-side, off the critical chain), so the drain tail
only carries PE/DVE/ACT ops plus the final output DMA.

Activation tables: Tanh and Sin co-reside only in silu_and_others; a Bacc
subclass pins table selection there so there is exactly one table load.
"""

import math
from contextlib import ExitStack

import numpy as np

import concourse.bass as bass
import concourse.tile as tile
from concourse import bacc, mybir
from concourse.hw_specs import get_activation_tables
import bass_rust as _bass_rust

F32 = mybir.dt.float32
F16 = mybir.dt.float16
I32 = mybir.dt.int32
AF = mybir.ActivationFunctionType
OP = mybir.AluOpType

N_CORES = 8
NROWS = 65536          # total rows
R = NROWS // N_CORES   # rows per core = 8192
CH = 512               # rows per chunk (one psum bank)
NCH = R // CH          # 16 chunks
NGRP = NCH // 4        # 4 groups
BLK = 2048             # xt dma block columns
NBLK = R // BLK        # 4 blocks

# optional fp16 lo-planes (one extra PE pass each where enabled)
LO_W1 = False
LO_W2 = False
LO_TW = False
LO_BS = False

MAGIC = float(np.float32(1.5 * 2 ** 23))   # fp32 round-to-nearest-int trick
TWO_PI = float(2.0 * math.pi)
LN54 = float(math.log(1.25))


def _wlayout():
    """Column layout of the packed fp16 weight tile."""
    off, lay = 0, {}
    for nm, w in (("w1h0", 128), ("w1h1", 128), ("w2h", 128), ("twh", 32),
                  ("bsh", 4)):
        lay[nm] = (off, off + w); off += w
    for flag, nm, w in ((LO_W1, "w1l0", 128), (LO_W1, "w1l1", 128),
                        (LO_W2, "w2l", 128), (LO_TW, "twl", 32),
                        (LO_BS, "bsl", 4)):
        if flag:
            lay[nm] = (off, off + w); off += w
    return lay, off


class _Bacc(bacc.Bacc):
    """Bacc whose activation-table pass may only pick silu_and_others
    (contains both Tanh and Sin) -> exactly one ACT_TABLE_LOAD."""

    def insert_act_table_loads(self):
        has_act = any(
            isinstance(i, mybir.InstActivation)
            for b in self.main_func.blocks
            for i in b.instructions
        )
        if not has_act:
            return
        tables = list(get_activation_tables(self.m.arch).items())
        masked = [
            (nm, fns if nm == "silu_and_others" else set()) for nm, fns in tables
        ]
        _bass_rust.insert_act_table_loads(self, masked)


def build_module():
    """Build + bacc-compile the (input-independent) Bass module."""
    lay, wcols = _wlayout()
    nc = _Bacc(
        "TRN2",
        target_bir_lowering=False,
        debug=False,
        enable_asserts=False,
        num_devices=N_CORES,
    )
    xh = nc.dram_tensor("xh", (2, 128, R), F16, kind="ExternalInput").ap()
    wpk = nc.dram_tensor("wpk", (128, wcols), F16, kind="ExternalInput").ap()
    # fincons cols: 0 = y scale (0.5*7/soa0), 1 = tanh bias, 2:6 = I4
    fincons = nc.dram_tensor("fincons", (128, 6), F32, kind="ExternalInput").ap()
    out = nc.dram_tensor("out", (R,), F32, kind="ExternalOutput").ap()

    with tile.TileContext(nc) as tc, ExitStack() as ctx:
        consts = ctx.enter_context(tc.tile_pool(name="consts", bufs=1))
        xpool = ctx.enter_context(tc.tile_pool(name="x", bufs=1))
        # PSUM budget (8 banks): z1 pairs 2x2 + z2 2 + u 2; the a-matmul and
        # its transposes reuse the drained u tile's partitions.
        z1ps = ctx.enter_context(tc.tile_pool(name="z1ps", bufs=2, space="PSUM"))
        z2ps = ctx.enter_context(tc.tile_pool(name="z2ps", bufs=2, space="PSUM"))
        ups = ctx.enter_context(tc.tile_pool(name="ups", bufs=2, space="PSUM"))
        t1p = ctx.enter_context(tc.tile_pool(name="t1p", bufs=2))
        t2p = ctx.enter_context(tc.tile_pool(name="t2p", bufs=3))
        kp = ctx.enter_context(tc.tile_pool(name="kp", bufs=2))
        vp = ctx.enter_context(tc.tile_pool(name="vp", bufs=2))
        shp = ctx.enter_context(tc.tile_pool(name="shp", bufs=2))
        rp = ctx.enter_context(tc.tile_pool(name="rp", bufs=2))
        finp = ctx.enter_context(tc.tile_pool(name="finp", bufs=1))

        wc = consts.tile([128, wcols], F16, tag="wc")
        W = {nm: wc[:, a:b] for nm, (a, b) in lay.items()}
        fcs = consts.tile([128, 6], F32, tag="fincons")
        ident4 = fcs[0:4, 2:6]
        # one x tile per block: plane k at cols [k*BLK, (k+1)*BLK)
        xt = [xpool.tile([128, 2 * BLK], F16, tag=f"xt{b}", name=f"xt{b}")
              for b in range(NBLK)]

        def xsl(c, k):
            return xt[c // 4][:, k * BLK + (c % 4) * CH:
                              k * BLK + (c % 4) * CH + CH]

        xv = xh.rearrange("k f c -> f k c")

        def xdst(b, c0, c1):
            return (xt[b][:].rearrange("f (k c) -> f k c", k=2)[:, :, c0:c1])

        # DMA transfers serialize on the shared DMA-engine pool, so issue
        # everything on ONE queue (SP) in strict need-order: per-chunk pieces
        # for the first two blocks, then the last two blocks whole. Weights
        # ride the gpsimd SWDGE queue so they win the first transfer slot.
        nc.gpsimd.dma_start(wc[:], wpk)
        for c in range(8):
            nc.sync.dma_start(xdst(c // 4, (c % 4) * CH, (c % 4 + 1) * CH),
                              xv[:, :, c * CH:(c + 1) * CH])
        nc.gpsimd.dma_start(fcs[:], fincons)
        nc.sync.dma_start(xdst(2, 0, BLK), xv[:, :, 2 * BLK:3 * BLK])
        nc.sync.dma_start(xdst(3, 0, BLK), xv[:, :, 3 * BLK:4 * BLK])

        # finals tiles (128, 64): partition 32g+8j+(r//64), col r%64 holds
        # group-g row 512j + r
        yfin = finp.tile([128, 64], F32, tag="yfin")
        afin = finp.tile([128, 64], F32, tag="afin")

        st = {}   # per-chunk state
        pr = {}   # per-pair state (z1/t1 are batched 2 chunks per tile)
        grp = {}  # per-group state

        def s_l1(c):
            p = c // 2
            st[c] = {}
            if p == 0:
                # first pair decoupled: separate psum tiles (borrowed from
                # the z2 pool rotation) so tanh1(c0) doesn't wait on chunk 1
                pr.setdefault(0, {})[f"z1s{c}"] = z2ps.tile(
                    [128, CH], F32, tag="z2", name=f"z1s_{c}")
                z1h = pr[0][f"z1s{c}"][:]
            else:
                if c % 2 == 0:
                    pr[p] = {"z1": z1ps.tile([128, 2 * CH], F32, tag="z1",
                                             name=f"z1_{p}")}
                z1h = pr[p]["z1"][:, (c % 2) * CH:(c % 2 + 1) * CH]
            passes = [W["w1h0"], W["w1h1"]]
            if LO_W1:
                passes += [W["w1l0"], W["w1l1"]]
            for i, w in enumerate(passes):
                nc.tensor.matmul(z1h, w, xsl(c, i % 2), start=(i == 0),
                                 stop=(i == len(passes) - 1))

        def s_tanh1(p, half=None):
            d = pr[p]
            if half is None:
                d["t1"] = t1p.tile([128, 2 * CH], F16, tag="t1", name=f"t1_{p}")
                nc.scalar.activation(d["t1"][:], d["z1"][:], AF.Tanh)
            else:
                if half == 0:
                    d["t1"] = t1p.tile([128, 2 * CH], F16, tag="t1",
                                       name=f"t1_{p}")
                sl = slice(half * CH, (half + 1) * CH)
                nc.scalar.activation(d["t1"][:, sl], d[f"z1s{half}"][:],
                                     AF.Tanh)

        def s_l2(c):
            d = st[c]
            t1h = pr[c // 2]["t1"][:, (c % 2) * CH:(c % 2 + 1) * CH]
            z2 = z2ps.tile([128, CH], F32, tag="z2", name=f"z2_{c}")
            d["z2"] = z2
            nc.tensor.matmul(z2[:], W["w2h"], t1h, start=True,
                             stop=not LO_W2)
            if LO_W2:
                nc.tensor.matmul(z2[:], W["w2l"], t1h, start=False,
                                 stop=True)

        def s_tanh2(c):
            d = st[c]
            d["t2"] = t2p.tile([128, CH], F16, tag="t2", name=f"t2_{c}")
            nc.scalar.activation(d["t2"][:], d["z2"][:], AF.Tanh)

        def s_umm(c):
            d = st[c]
            g, j = c // 4, c % 4
            if j == 0:
                grp[g] = {"u": ups.tile([128, CH], F32, tag="u",
                                        name=f"u_{g}")}
            od = grp[g]["u"][32 * j:32 * (j + 1), :]
            tp = (0, 32 * j)
            nc.tensor.matmul(od, W["twh"], d["t2"][:], start=True,
                             stop=not LO_TW, tile_position=tp)
            if LO_TW:
                nc.tensor.matmul(od, W["twl"], d["t2"][:], start=False,
                                 stop=True, tile_position=tp)
            del st[c]

        def s_ga(g):
            d = grp[g]
            # u drain for the y rows {0,32,64,96}; the last group uses the
            # (then idle) ACT engine so DVE can start k/v immediately
            d["uc"] = rp.tile([128, CH], F32, tag="uc", name=f"uc_{g}")
            if g == NGRP - 1:
                nc.scalar.copy(d["uc"][:], d["u"][:])
            else:
                nc.vector.tensor_copy(d["uc"][:], d["u"][:])
            # j-major strip of the 4 y rows straight into yfin's group block
            raw4 = d["uc"][:].rearrange("(jj h) r -> h jj r", h=32)[0]
            nc.sync.dma_start(yfin[32 * g:32 * (g + 1), :], raw4)
            # range reduction: k = rint(u) (MAGIC trick), v = u - k
            d["k"] = kp.tile([128, CH], F32, tag="k", name=f"k_{g}")
            nc.vector.tensor_scalar(d["k"][:], d["u"][:], MAGIC, -MAGIC,
                                    OP.add, OP.add)
            d["v"] = vp.tile([128, CH], F32, tag="v", name=f"v_{g}")
            nc.vector.scalar_tensor_tensor(d["v"][:], d["u"][:], 0.0,
                                           d["k"][:], OP.add, OP.subtract)

        def s_gb(g):
            d = grp[g]
            d["sh"] = shp.tile([128, CH], F16, tag="sh", name=f"sh_{g}")
            nc.scalar.activation(d["sh"][:], d["v"][:], AF.Sin, scale=TWO_PI)

        def s_gc(g):
            # u is fully drained: reuse it for the a matmul (partitions 0:4)
            d = grp[g]
            a4 = d["u"][0:4, :]
            d["a4"] = a4
            nc.tensor.matmul(a4, W["bsh"], d["sh"][:], start=True,
                             stop=not LO_BS)
            if LO_BS:
                nc.tensor.matmul(a4, W["bsl"], d["sh"][:], start=False,
                                 stop=True)

        def s_gd(g):
            d = grp[g]
            ar4 = rp.tile([4, CH], F32, tag="ar4", name=f"ar4_{g}")
            nc.vector.tensor_copy(ar4[:], d["a4"])
            nc.sync.dma_start(afin[32 * g:32 * (g + 1), :], ar4[:])
            del grp[g]

        def s_fin():
            """Batched finals on the gathered (128, 64) tiles + output DMA.
            Critical chain after the last afin gather: aab -> tnh -> ot."""
            t = {}
            for nm, dt_ in (("aab", I32), ("gsn", I32), ("tnh", F32),
                            ("yv", F32), ("ysg", I32), ("ot", F32)):
                t[nm] = finp.tile([128, 64], dt_, tag=nm, name=nm)
            # off-chain ops first (only depend on yfin / afin sign bits)
            nc.vector.tensor_scalar(t["yv"][:], yfin[:], fcs[:, 0:1],
                                    None, OP.mult)
            nc.vector.tensor_scalar(t["gsn"][:], afin[:].bitcast(I32),
                                    -2 ** 31, None, OP.bitwise_and)
            nc.vector.tensor_tensor(t["ysg"][:], t["yv"][:].bitcast(I32),
                                    t["gsn"][:], OP.bitwise_xor)
            nc.vector.tensor_scalar(t["aab"][:], afin[:].bitcast(I32),
                                    0x7FFFFFFF, None, OP.bitwise_and)
            nc.scalar.activation(t["tnh"][:], t["aab"][:].bitcast(F32),
                                 AF.Tanh, scale=0.5, bias=fcs[:, 1:2])
            # ot = (tnh + 1) * ysg  == sign(a) * y * sigmoid(|a|-ln(5/4))
            nc.vector.scalar_tensor_tensor(t["ot"][:], t["tnh"][:], 1.0,
                                           t["ysg"][:].bitcast(F32),
                                           OP.add, OP.mult)
            nc.sync.dma_start(out.rearrange("(a b) -> a b", b=64),
                              t["ot"][:])

        # modulo schedule: emission order fixes per-engine FIFO order
        for t in range(NCH + 10):
            if t < NCH:
                s_l1(t)
            if t in (1, 2):      # pair 0 unpaired for fast start
                s_tanh1(0, half=t - 1)
            if t % 2 == 0 and 4 <= t <= NCH:
                s_tanh1(t // 2 - 1)
            c = t - 3
            if 0 <= c < NCH:
                s_l2(c)
            c = t - 4
            if 0 <= c < NCH:
                s_tanh2(c)
            c = t - 5
            if 0 <= c < NCH:
                s_umm(c)
            c = t - 6   # c%4==3 completes group g=c//4
            if 0 <= c < NCH and c % 4 == 3:
                s_ga(c // 4)
            c = t - 7
            if 0 <= c < NCH and c % 4 == 3:
                s_gb(c // 4)
            c = t - 8
            if 0 <= c < NCH and c % 4 == 3:
                s_gc(c // 4)
            c = t - 9
            if 0 <= c < NCH and c % 4 == 3:
                s_gd(c // 4)
        s_fin()

    nc.compile()
    return nc


_NC_CACHE = None


def _get_module():
    global _NC_CACHE
    if _NC_CACHE is None:
        _NC_CACHE = build_module()
    return _NC_CACHE


def _f16(v):
    return np.asarray(v, np.float32).astype(np.float16)


def _f16lo(v):
    v = np.asarray(v, np.float32)
    h = v.astype(np.float16)
    return (v - h.astype(np.float32)).astype(np.float16)


def prep_inputs(x, W1, W2, Wout, s1a, s1b, s2a, s2b, soa, sob):
    """Host-side prep: shard x^T per core (fp16), prepack weights."""
    lay, wcols = _wlayout()
    x = np.asarray(x, np.float32).reshape(NROWS, 256)
    W1 = np.asarray(W1, np.float64)
    W2 = np.asarray(W2, np.float64)
    wout = np.asarray(Wout, np.float64)[0]          # (128,)
    soa_v = np.asarray(soa, np.float64)[:, 0]       # (32,)
    sob_v = np.asarray(sob, np.float64)[0]          # (32,)

    # component order: y-recovery component first (a is order-invariant)
    hstar = int(np.argmax(np.abs(soa_v)))
    perm = [hstar] + [h for h in range(32) if h != hstar]
    soa_p = soa_v[perm]
    sob_p = sob_v[perm]

    w1t = np.ascontiguousarray(
        W1.reshape(128, 2, 128).transpose(1, 2, 0))     # (2,128,128) [k,f,m]
    w2t = np.ascontiguousarray(W2.T)
    tailw = np.ascontiguousarray(
        wout[:, None] * soa_p[None, :] / 7.0)           # (128,32)
    bsob = np.zeros((128, 4), np.float64)
    for j in range(4):
        bsob[32 * j:32 * (j + 1), j] = sob_p

    wpk = np.zeros((128, wcols), np.float16)
    parts = {"w1h0": _f16(w1t[0]), "w1h1": _f16(w1t[1]),
             "w2h": _f16(w2t), "twh": _f16(tailw), "bsh": _f16(bsob)}
    if LO_W1:
        parts["w1l0"] = _f16lo(w1t[0]); parts["w1l1"] = _f16lo(w1t[1])
    if LO_W2:
        parts["w2l"] = _f16lo(w2t)
    if LO_TW:
        parts["twl"] = _f16lo(tailw)
    if LO_BS:
        parts["bsl"] = _f16lo(bsob)
    for nm, (a, b) in lay.items():
        wpk[:, a:b] = parts[nm]

    # col0: y-recovery scale (0.5 from sigmoid=0.5*(1+tanh) folded in)
    # col1: tanh bias -ln(5/4)/2; cols 2:6 identity for PE transposes
    fincons = np.zeros((128, 6), np.float32)
    fincons[:, 0] = np.float32(0.5 * 7.0 / soa_p[0])
    fincons[:, 1] = np.float32(-LN54 / 2.0)
    fincons[0:4, 2:6] = np.eye(4, dtype=np.float32)

    xT = x.T.astype(np.float16)                       # (256, 65536)
    in_maps = []
    for c in range(N_CORES):
        xc = np.ascontiguousarray(xT[:, c * R:(c + 1) * R]).reshape(2, 128, R)
        in_maps.append({
            "xh": xc, "wpk": wpk, "fincons": fincons,
        })
    return in_maps


def kernel(x, W1, W2, Wout, s1a, s1b, s2a, s2b, soa, sob):
    from concourse.bass_utils import run_bass_kernel_spmd

    nc = _get_module()
    in_maps = prep_inputs(x, W1, W2, Wout, s1a, s1b, s2a, s2b, soa, sob)
    res = run_bass_kernel_spmd(nc, in_maps, core_ids=list(range(N_CORES)))
    full = np.concatenate([res.results[c]["out"] for c in range(N_CORES)])
    return full.reshape(1024, 64).astype(np.float32)


# revision 40
# speedup vs baseline: 1.1295x; 1.0183x over previous
"""Trainium2 Bass kernel for nn_Dataset1V7Table5Redo_69741678952822 (topk_masking).

Math: the reference's set-valued +/- path expansion collapses algebraically.
Per row (N = batch*choices = 65536, D = 256):
    t1 = tanh(W1 @ x)            (128)
    t2 = tanh(W2 @ t1)           (128)
    y  = Wout @ t2               (scalar)
    a  = sum_h sob[h] * sin(2*pi*soa[h]*y/7)
    out = sign(a) * y * sigmoid(|a| - ln(5/4))

Sharding: pure data parallel over rows, 8192 rows/core on 8 cores.
Host pre-transposes x so the contraction dim lands on SBUF partitions.

Precision: single-fp16 everywhere (x, weights, activations). Host-side
float64 simulation of this exact scheme gives rel err 5.5e-4 vs the fp32
reference (gate is 2e-2): the handful of sign(a) flips land near y=0 where
the output is tiny. Optional fp16 hi/lo planes per weight matrix can be
re-enabled via the LO_* flags (each adds one PE pass per matmul).

Structure per 512-row chunk: PE does 2 L1 k-half passes + 1 L2 + 1 u pass
(+ a/4); ACT does tanh1 (paired across 2 chunks to amortize access bubbles),
tanh2 and sin/4, each writing fp16 SBUF directly. Group tails (2048 rows):
rint range reduction (MAGIC trick) on DVE, sin on ACT, then the per-row 'a'
reduction on PE. The (4,512) a / y rows are reshaped into a c-major (64,128)
finals layout with 8 tiny PE transposes per group (a-side, no DMA) and one
strided SBUF gather DMA (y-side, off the critical chain), so the drain tail
only carries PE/DVE/ACT ops plus the final output DMA.

Activation tables: Tanh and Sin co-reside only in silu_and_others; a Bacc
subclass pins table selection there so there is exactly one table load.
"""

import math
from contextlib import ExitStack

import numpy as np

import concourse.bass as bass
import concourse.tile as tile
from concourse import bacc, mybir
from concourse.hw_specs import get_activation_tables
import bass_rust as _bass_rust

F32 = mybir.dt.float32
F16 = mybir.dt.float16
I32 = mybir.dt.int32
AF = mybir.ActivationFunctionType
OP = mybir.AluOpType

N_CORES = 8
NROWS = 65536          # total rows
R = NROWS // N_CORES   # rows per core = 8192
CH = 512               # rows per chunk (one psum bank)
NCH = R // CH          # 16 chunks
NGRP = NCH // 4        # 4 groups
BLK = 2048             # xt dma block columns
NBLK = R // BLK        # 4 blocks

# optional fp16 lo-planes (one extra PE pass each where enabled)
LO_W1 = False
LO_W2 = False
LO_TW = False
LO_BS = False

MAGIC = float(np.float32(1.5 * 2 ** 23))   # fp32 round-to-nearest-int trick
TWO_PI = float(2.0 * math.pi)
LN54 = float(math.log(1.25))


def _wlayout():
    """Column layout of the packed fp16 weight tile."""
    off, lay = 0, {}
    for nm, w in (("w1h0", 128), ("w1h1", 128), ("w2h", 128), ("twh", 32),
                  ("bsh", 4), ("wsel", 256)):
        lay[nm] = (off, off + w); off += w
    for flag, nm, w in ((LO_W1, "w1l0", 128), (LO_W1, "w1l1", 128),
                        (LO_W2, "w2l", 128), (LO_TW, "twl", 32),
                        (LO_BS, "bsl", 4)):
        if flag:
            lay[nm] = (off, off + w); off += w
    return lay, off


class _Bacc(bacc.Bacc):
    """Bacc whose activation-table pass may only pick silu_and_others
    (contains both Tanh and Sin) -> exactly one ACT_TABLE_LOAD."""

    def insert_act_table_loads(self):
        has_act = any(
            isinstance(i, mybir.InstActivation)
            for b in self.main_func.blocks
            for i in b.instructions
        )
        if not has_act:
            return
        tables = list(get_activation_tables(self.m.arch).items())
        masked = [
            (nm, fns if nm == "silu_and_others" else set()) for nm, fns in tables
        ]
        _bass_rust.insert_act_table_loads(self, masked)


def build_module():
    """Build + bacc-compile the (input-independent) Bass module."""
    lay, wcols = _wlayout()
    nc = _Bacc(
        "TRN2",
        target_bir_lowering=False,
        debug=False,
        enable_asserts=False,
        num_devices=N_CORES,
    )
    xh = nc.dram_tensor("xh", (2, 128, R), F16, kind="ExternalInput").ap()
    wpk = nc.dram_tensor("wpk", (128, wcols), F16, kind="ExternalInput").ap()
    # fincons cols: 0 = y scale (0.5*7/soa0), 1 = tanh bias, 2:6 = I4
    fincons = nc.dram_tensor("fincons", (128, 6), F32, kind="ExternalInput").ap()
    out = nc.dram_tensor("out", (R,), F32, kind="ExternalOutput").ap()

    with tile.TileContext(nc) as tc, ExitStack() as ctx:
        consts = ctx.enter_context(tc.tile_pool(name="consts", bufs=1))
        xpool = ctx.enter_context(tc.tile_pool(name="x", bufs=1))
        # PSUM budget (8 banks): z1 pairs 2x2 + z2 2 + u 2; the a-matmul and
        # its transposes reuse the drained u tile's partitions.
        z1ps = ctx.enter_context(tc.tile_pool(name="z1ps", bufs=2, space="PSUM"))
        z2ps = ctx.enter_context(tc.tile_pool(name="z2ps", bufs=2, space="PSUM"))
        ups = ctx.enter_context(tc.tile_pool(name="ups", bufs=2, space="PSUM"))
        t1p = ctx.enter_context(tc.tile_pool(name="t1p", bufs=2))
        t2p = ctx.enter_context(tc.tile_pool(name="t2p", bufs=3))
        kp = ctx.enter_context(tc.tile_pool(name="kp", bufs=2))
        vp = ctx.enter_context(tc.tile_pool(name="vp", bufs=2))
        shp = ctx.enter_context(tc.tile_pool(name="shp", bufs=2))
        rp = ctx.enter_context(tc.tile_pool(name="rp", bufs=2))
        finp = ctx.enter_context(tc.tile_pool(name="finp", bufs=1))

        wc = consts.tile([128, wcols], F16, tag="wc")
        W = {nm: wc[:, a:b] for nm, (a, b) in lay.items()}
        fcs = consts.tile([128, 6], F32, tag="fincons")
        ident4 = fcs[0:4, 2:6]
        # one x tile per block: plane k at cols [k*BLK, (k+1)*BLK)
        xt = [xpool.tile([128, 2 * BLK], F16, tag=f"xt{b}", name=f"xt{b}")
              for b in range(NBLK)]

        def xsl(c, k):
            return xt[c // 4][:, k * BLK + (c % 4) * CH:
                              k * BLK + (c % 4) * CH + CH]

        xv = xh.rearrange("k f c -> f k c")

        def xdst(b, c0, c1):
            return (xt[b][:].rearrange("f (k c) -> f k c", k=2)[:, :, c0:c1])

        # DMA transfers serialize on the shared DMA-engine pool, so issue
        # everything on ONE queue (SP) in strict need-order: per-chunk pieces
        # for the first two blocks, then the last two blocks whole. Weights
        # ride the gpsimd SWDGE queue so they win the first transfer slot.
        nc.gpsimd.dma_start(wc[:], wpk)
        for c in range(8):
            nc.sync.dma_start(xdst(c // 4, (c % 4) * CH, (c % 4 + 1) * CH),
                              xv[:, :, c * CH:(c + 1) * CH])
        nc.gpsimd.dma_start(fcs[:], fincons)
        nc.sync.dma_start(xdst(2, 0, BLK), xv[:, :, 2 * BLK:3 * BLK])
        nc.sync.dma_start(xdst(3, 0, BLK), xv[:, :, 3 * BLK:4 * BLK])

        # finals tiles (128, 64): partition 32g+8j+(r//64), col r%64 holds
        # group-g row 512j + r
        yfin = finp.tile([128, 64], F32, tag="yfin")
        afin = finp.tile([128, 64], F32, tag="afin")

        st = {}   # per-chunk state
        pr = {}   # per-pair state (z1/t1 are batched 2 chunks per tile)
        grp = {}  # per-group state

        def s_l1(c):
            p = c // 2
            st[c] = {}
            if p == 0:
                # first pair decoupled: separate psum tiles (borrowed from
                # the z2 pool rotation) so tanh1(c0) doesn't wait on chunk 1
                pr.setdefault(0, {})[f"z1s{c}"] = z2ps.tile(
                    [128, CH], F32, tag="z2", name=f"z1s_{c}")
                z1h = pr[0][f"z1s{c}"][:]
            else:
                if c % 2 == 0:
                    pr[p] = {"z1": z1ps.tile([128, 2 * CH], F32, tag="z1",
                                             name=f"z1_{p}")}
                z1h = pr[p]["z1"][:, (c % 2) * CH:(c % 2 + 1) * CH]
            passes = [W["w1h0"], W["w1h1"]]
            if LO_W1:
                passes += [W["w1l0"], W["w1l1"]]
            for i, w in enumerate(passes):
                nc.tensor.matmul(z1h, w, xsl(c, i % 2), start=(i == 0),
                                 stop=(i == len(passes) - 1))

        def s_tanh1(p, half=None):
            d = pr[p]
            if half is None:
                d["t1"] = t1p.tile([128, 2 * CH], F16, tag="t1", name=f"t1_{p}")
                nc.scalar.activation(d["t1"][:], d["z1"][:], AF.Tanh)
            else:
                if half == 0:
                    d["t1"] = t1p.tile([128, 2 * CH], F16, tag="t1",
                                       name=f"t1_{p}")
                sl = slice(half * CH, (half + 1) * CH)
                nc.scalar.activation(d["t1"][:, sl], d[f"z1s{half}"][:],
                                     AF.Tanh)

        def s_l2(c):
            d = st[c]
            t1h = pr[c // 2]["t1"][:, (c % 2) * CH:(c % 2 + 1) * CH]
            z2 = z2ps.tile([128, CH], F32, tag="z2", name=f"z2_{c}")
            d["z2"] = z2
            nc.tensor.matmul(z2[:], W["w2h"], t1h, start=True,
                             stop=not LO_W2)
            if LO_W2:
                nc.tensor.matmul(z2[:], W["w2l"], t1h, start=False,
                                 stop=True)

        def s_tanh2(c):
            d = st[c]
            d["t2"] = t2p.tile([128, CH], F16, tag="t2", name=f"t2_{c}")
            nc.scalar.activation(d["t2"][:], d["z2"][:], AF.Tanh)

        def s_umm(c):
            d = st[c]
            g, j = c // 4, c % 4
            if j == 0:
                grp[g] = {"u": ups.tile([128, CH], F32, tag="u",
                                        name=f"u_{g}")}
            od = grp[g]["u"][32 * j:32 * (j + 1), :]
            tp = (0, 32 * j)
            nc.tensor.matmul(od, W["twh"], d["t2"][:], start=True,
                             stop=not LO_TW, tile_position=tp)
            if LO_TW:
                nc.tensor.matmul(od, W["twl"], d["t2"][:], start=False,
                                 stop=True, tile_position=tp)
            del st[c]

        def s_ga(g):
            d = grp[g]
            # u drain (fp16) for the y rows {0,32,64,96}; the last group uses
            # the (then idle) ACT engine so DVE can start k/v immediately
            d["uc"] = rp.tile([128, CH], F16, tag="uc", name=f"uc_{g}")
            if g == NGRP - 1:
                nc.scalar.copy(d["uc"][:], d["u"][:])
            else:
                nc.vector.tensor_copy(d["uc"][:], d["u"][:])
            # j-major strip of the 4 y rows; scattered into yfin in s_ge
            raw4 = d["uc"][:].rearrange("(jj h) r -> h jj r", h=32)[0]
            d["yr4"] = rp.tile([4, CH], F16, tag="yr4", name=f"yr4_{g}")
            nc.sync.dma_start(d["yr4"][:], raw4)
            # range reduction: k = rint(u) (MAGIC trick), v = u - k
            d["k"] = kp.tile([128, CH], F32, tag="k", name=f"k_{g}")
            nc.vector.tensor_scalar(d["k"][:], d["u"][:], MAGIC, -MAGIC,
                                    OP.add, OP.add)
            d["v"] = vp.tile([128, CH], F32, tag="v", name=f"v_{g}")
            nc.vector.scalar_tensor_tensor(d["v"][:], d["u"][:], 0.0,
                                           d["k"][:], OP.add, OP.subtract)

        def s_gb(g):
            d = grp[g]
            d["sh"] = shp.tile([128, CH], F16, tag="sh", name=f"sh_{g}")
            nc.scalar.activation(d["sh"][:], d["v"][:], AF.Sin, scale=TWO_PI)

        def s_gc(g):
            # u is fully drained: reuse it for the a matmul (partitions 0:4)
            d = grp[g]
            a4 = d["u"][0:4, :]
            d["a4"] = a4
            nc.tensor.matmul(a4, W["bsh"], d["sh"][:], start=True,
                             stop=not LO_BS)
            if LO_BS:
                nc.tensor.matmul(a4, W["bsl"], d["sh"][:], start=False,
                                 stop=True)

        def scatter4(src4, dst_region, dst_fin, g):
            """Selector-matmul scatter: (4,512) fp16 row-major -> (32,64)
            j-major psum block (partition 8j+q, col c <- row 512j+64q+c),
            then one cheap DVE copy into the (128,64) finals tile."""
            for q in range(8):
                nc.tensor.matmul(dst_region, W["wsel"][0:4, 32 * q:32 * (q + 1)],
                                 src4[0:4, 64 * q:64 * (q + 1)],
                                 start=(q == 0), stop=(q == 7))
            nc.vector.tensor_copy(dst_fin[32 * g:32 * (g + 1), :], dst_region)

        def s_gd(g):
            d = grp[g]
            # a4 -> fp16 SBUF (sign-exact), then scatter into afin via PE
            ar4 = rp.tile([4, CH], F16, tag="ar4", name=f"ar4_{g}")
            if g == NGRP - 1:
                nc.scalar.copy(ar4[:], d["a4"])
            else:
                nc.vector.tensor_copy(ar4[:], d["a4"])
            scatter4(ar4, d["u"][0:32, 0:64], afin, g)

        def s_ge(g):
            # y-side scatter; emitted a few ticks later so the PE FIFO never
            # waits on the yr4 strip DMA
            d = grp[g]
            scatter4(d["yr4"], d["u"][0:32, 64:128], yfin, g)
            del grp[g]

        def s_fin():
            """Batched finals on the gathered (128, 64) tiles + output DMA.
            Critical chain after the last afin gather: aab -> tnh -> ot."""
            t = {}
            for nm, dt_ in (("aab", I32), ("gsn", I32), ("tnh", F32),
                            ("yv", F32), ("ysg", I32), ("ot", F32)):
                t[nm] = finp.tile([128, 64], dt_, tag=nm, name=nm)
            # off-chain ops first (only depend on yfin / afin sign bits)
            nc.vector.tensor_scalar(t["yv"][:], yfin[:], fcs[:, 0:1],
                                    None, OP.mult)
            nc.vector.tensor_scalar(t["gsn"][:], afin[:].bitcast(I32),
                                    -2 ** 31, None, OP.bitwise_and)
            nc.vector.tensor_tensor(t["ysg"][:], t["yv"][:].bitcast(I32),
                                    t["gsn"][:], OP.bitwise_xor)
            nc.vector.tensor_scalar(t["aab"][:], afin[:].bitcast(I32),
                                    0x7FFFFFFF, None, OP.bitwise_and)
            nc.scalar.activation(t["tnh"][:], t["aab"][:].bitcast(F32),
                                 AF.Tanh, scale=0.5, bias=fcs[:, 1:2])
            # ot = (tnh + 1) * ysg  == sign(a) * y * sigmoid(|a|-ln(5/4))
            nc.vector.scalar_tensor_tensor(t["ot"][:], t["tnh"][:], 1.0,
                                           t["ysg"][:].bitcast(F32),
                                           OP.add, OP.mult)
            nc.sync.dma_start(out.rearrange("(a b) -> a b", b=64),
                              t["ot"][:])

        # modulo schedule: emission order fixes per-engine FIFO order
        for t in range(NCH + 10):
            if t < NCH:
                s_l1(t)
            if t in (1, 2):      # pair 0 unpaired for fast start
                s_tanh1(0, half=t - 1)
            if t % 2 == 0 and 4 <= t <= NCH:
                s_tanh1(t // 2 - 1)
            c = t - 3
            if 0 <= c < NCH:
                s_l2(c)
            c = t - 4
            if 0 <= c < NCH:
                s_tanh2(c)
            c = t - 10  # before s_umm: u_{g+2} realloc must follow s_ge(g)
            if 0 <= c < NCH and c % 4 == 3:
                s_ge(c // 4)
            c = t - 5
            if 0 <= c < NCH:
                s_umm(c)
            c = t - 6   # c%4==3 completes group g=c//4
            if 0 <= c < NCH and c % 4 == 3:
                s_ga(c // 4)
            c = t - 7
            if 0 <= c < NCH and c % 4 == 3:
                s_gb(c // 4)
            c = t - 8
            if 0 <= c < NCH and c % 4 == 3:
                s_gc(c // 4)
            c = t - 9
            if 0 <= c < NCH and c % 4 == 3:
                s_gd(c // 4)
        s_fin()

    nc.compile()
    return nc


_NC_CACHE = None


def _get_module():
    global _NC_CACHE
    if _NC_CACHE is None:
        _NC_CACHE = build_module()
    return _NC_CACHE


def _f16(v):
    return np.asarray(v, np.float32).astype(np.float16)


def _f16lo(v):
    v = np.asarray(v, np.float32)
    h = v.astype(np.float16)
    return (v - h.astype(np.float32)).astype(np.float16)


def prep_inputs(x, W1, W2, Wout, s1a, s1b, s2a, s2b, soa, sob):
    """Host-side prep: shard x^T per core (fp16), prepack weights."""
    lay, wcols = _wlayout()
    x = np.asarray(x, np.float32).reshape(NROWS, 256)
    W1 = np.asarray(W1, np.float64)
    W2 = np.asarray(W2, np.float64)
    wout = np.asarray(Wout, np.float64)[0]          # (128,)
    soa_v = np.asarray(soa, np.float64)[:, 0]       # (32,)
    sob_v = np.asarray(sob, np.float64)[0]          # (32,)

    # component order: y-recovery component first (a is order-invariant)
    hstar = int(np.argmax(np.abs(soa_v)))
    perm = [hstar] + [h for h in range(32) if h != hstar]
    soa_p = soa_v[perm]
    sob_p = sob_v[perm]

    w1t = np.ascontiguousarray(
        W1.reshape(128, 2, 128).transpose(1, 2, 0))     # (2,128,128) [k,f,m]
    w2t = np.ascontiguousarray(W2.T)
    tailw = np.ascontiguousarray(
        wout[:, None] * soa_p[None, :] / 7.0)           # (128,32)
    bsob = np.zeros((128, 4), np.float64)
    for j in range(4):
        bsob[32 * j:32 * (j + 1), j] = sob_p

    # selector matrices for the (4,512)->(32,64) j-major PE scatter
    wsel = np.zeros((128, 256), np.float64)
    for q in range(8):
        for j in range(4):
            wsel[j, 32 * q + 8 * j + q] = 1.0

    wpk = np.zeros((128, wcols), np.float16)
    parts = {"w1h0": _f16(w1t[0]), "w1h1": _f16(w1t[1]),
             "w2h": _f16(w2t), "twh": _f16(tailw), "bsh": _f16(bsob),
             "wsel": _f16(wsel)}
    if LO_W1:
        parts["w1l0"] = _f16lo(w1t[0]); parts["w1l1"] = _f16lo(w1t[1])
    if LO_W2:
        parts["w2l"] = _f16lo(w2t)
    if LO_TW:
        parts["twl"] = _f16lo(tailw)
    if LO_BS:
        parts["bsl"] = _f16lo(bsob)
    for nm, (a, b) in lay.items():
        wpk[:, a:b] = parts[nm]

    # col0: y-recovery scale (0.5 from sigmoid=0.5*(1+tanh) folded in)
    # col1: tanh bias -ln(5/4)/2; cols 2:6 identity for PE transposes
    fincons = np.zeros((128, 6), np.float32)
    fincons[:, 0] = np.float32(0.5 * 7.0 / soa_p[0])
    fincons[:, 1] = np.float32(-LN54 / 2.0)
    fincons[0:4, 2:6] = np.eye(4, dtype=np.float32)

    xT = x.T.astype(np.float16)                       # (256, 65536)
    in_maps = []
    for c in range(N_CORES):
        xc = np.ascontiguousarray(xT[:, c * R:(c + 1) * R]).reshape(2, 128, R)
        in_maps.append({
            "xh": xc, "wpk": wpk, "fincons": fincons,
        })
    return in_maps


def kernel(x, W1, W2, Wout, s1a, s1b, s2a, s2b, soa, sob):
    from concourse.bass_utils import run_bass_kernel_spmd

    nc = _get_module()
    in_maps = prep_inputs(x, W1, W2, Wout, s1a, s1b, s2a, s2b, soa, sob)
    res = run_bass_kernel_spmd(nc, in_maps, core_ids=list(range(N_CORES)))
    full = np.concatenate([res.results[c]["out"] for c in range(N_CORES)])
    return full.reshape(1024, 64).astype(np.float32)
